# revision 20
# baseline (speedup 1.0000x reference)
"""AttentionMIL pooling kernel for 8 Trainium2 NeuronCores.

Math (per slide b): h = tanh(X @ W1^T); s = h @ w2; a = softmax(s);
out = a^T @ X, with X [N=8192, D=1024], W1 [H=256, D], w2 [H].

Strategy (v2 — single X copy, wsum on the vector engine):
  - Data-parallel over the slide dim: 16 slides / 8 cores = 2 per core.
  - ONE host-swizzled transposed copy of X per core (bf16, [128(d-chunk),
    n-free] tiles) — 32 MiB of HBM traffic per core instead of the 64 MiB
    the two-layout variant needed.
  - Scores in h^T orientation: for each 512-row n-tile, PE computes
    hT[half] [128, 512] = w1t_chunk^T @ xt_chunk accumulated over the 8
    d-chunks (N=512 moving operand — best PE efficiency), ACT applies
    tanh -> bf16, then two more PE matmuls with a REPLICATED w2 stationary
    ([128, 128] with every column equal to w2-half) produce the scores
    already broadcast across all 128 partitions; ACT exp -> e128 bf16.
  - Softmax without a max pass: s = w2 . tanh(.) is bounded by ||w2||_1
    (~13 for this data), so exp(s) cannot overflow fp32.
  - Weighted sum OFF the tensor engine: DVE tensor_tensor_reduce computes
    acc[d-chunk] += sum_n xt[d, n] * e128[d, n] per tile (fused multiply+
    reduce over the free dim, one bf16 2x-mode pass), accumulating into
    per-tile columns of an SBUF tile; one final reduce folds the 16 tile
    columns. l = sum(e) via a 1-lane reduce of e128 row 0 per tile.
  - out = acc / l on host (tiny).
"""

import sys

sys.path.insert(0, "/opt/trn_rl_repo")

import numpy as np
import ml_dtypes

import concourse.bacc as bacc
import concourse.tile as tile
from concourse import mybir
from concourse.bass_utils import run_bass_kernel_spmd

BF16 = ml_dtypes.bfloat16
B, N, D, H = 16, 8192, 1024, 256
NCORES = 8
SPC = B // NCORES          # slides per core
NT = 512                   # rows of N per tile
TILES = N // NT
KCH = D // 128             # d-chunks (contraction chunks of 128)
HH = H // 128              # h halves
GP_CHUNKS = 2              # d-chunks routed GpSimd(mul) + Scalar(accum)

_NC_CACHE = {}


def _build_nc():
    bf = mybir.dt.bfloat16
    f32 = mybir.dt.float32
    AF = mybir.ActivationFunctionType
    OP = mybir.AluOpType

    nc = bacc.Bacc("TRN2", num_devices=NCORES)
    # Host-swizzled transposed layout: each per-tile DMA reads one fully
    # contiguous 1 MiB region into a [128, free] SBUF tile.
    #   xt[s, t, p, k*NT + j] = X[s, t*NT + j, k*128 + p]
    xt = nc.declare_dram_parameter("xt", [SPC, TILES, 128, KCH * NT], bf, isOutput=False)
    # w1t[p, k*H + h] = W1[h, k*128 + p]
    w1t = nc.declare_dram_parameter("w1t", [128, KCH * H], bf, isOutput=False)
    # w2rep[p, half*128 + c] = W2[0, half*128 + p]  (replicated along c)
    w2rep = nc.declare_dram_parameter("w2rep", [128, H], bf, isOutput=False)
    outp = nc.declare_dram_parameter("out", [SPC, 128, KCH], f32, isOutput=True)
    # e row per tile, summed on host for the softmax denominator
    oute = nc.declare_dram_parameter("oute", [SPC, TILES, NT], bf, isOutput=True)

    with tile.TileContext(nc) as tc:
        with tc.tile_pool(name="const", bufs=1) as constp, \
             tc.tile_pool(name="xt", bufs=6) as xtp, \
             tc.tile_pool(name="tanh", bufs=3) as tp, \
             tc.tile_pool(name="e128", bufs=3) as ep, \
             tc.tile_pool(name="scr", bufs=2) as scrp, \
             tc.tile_pool(name="scra", bufs=2) as scrap, \
             tc.tile_pool(name="gprod", bufs=3) as gprodp, \
             tc.tile_pool(name="racc", bufs=2) as raccp, \
             tc.tile_pool(name="outsb", bufs=2) as outsbp, \
             tc.tile_pool(name="hps", bufs=2, space="PSUM") as hpsp, \
             tc.tile_pool(name="sps", bufs=2, space="PSUM") as spsp, \
             tc.tile_pool(name="warm", bufs=1, space="PSUM") as warmp:

            w1t_sb = constp.tile([128, KCH * H], bf)
            nc.gpsimd.dma_start(w1t_sb[:], w1t[:, :])
            w2r_sb = constp.tile([128, H], bf)
            nc.gpsimd.dma_start(w2r_sb[:], w2rep[:, :])

            warm_sb = constp.tile([128, 256], bf)
            nc.gpsimd.memset(warm_sb[:], 0.0)
            warm_ps = warmp.tile([128, 256], f32)
            for _ in range(28):
                nc.tensor.matmul(
                    warm_ps[:, 0:H], warm_sb[:, 0:128], warm_sb[:, 0:H],
                    start=True, stop=True, skip_group_check=True,
                )

            state = {}          # per-slide persistent tiles
            prev = None         # (s, t, xt_sb, tanh_sb)
            prev_gp = None      # (s, t, prod_g)

            def emit_scores_and_wsum(s, t, xt_sb, tanh_sb):
                # scores: two matmuls with replicated-w2 stationary ->
                # s_ps [128, 512] (every partition = the score row)
                s_ps = spsp.tile([128, NT], f32)
                for half in range(HH):
                    nc.tensor.matmul(
                        s_ps[:],
                        w2r_sb[:, half * 128:(half + 1) * 128],
                        tanh_sb[:, half * NT:(half + 1) * NT],
                        start=(half == 0), stop=(half == HH - 1),
                    )
                e_sb = ep.tile([128, NT], bf)
                nc.scalar.activation(e_sb[:], s_ps[:], AF.Exp)
                nc.sync.dma_start(oute[s, t:t + 1, :], e_sb[0:1, :])

                racc = state[s]
                scr = scrp.tile([128, NT], bf)
                # Fused STT runs at 1x on DVE (bf16 2x is stock-op only), so
                # DVE alone (8 x 613ns) can't keep up with PE (3.84us/tile).
                # Ship GP_CHUNKS d-chunks to GpSimd (multiply) + Scalar
                # (Copy-with-accum reduce); DVE keeps the rest fused.
                prod_g = gprodp.tile([128, GP_CHUNKS * NT], bf)
                for k in range(GP_CHUNKS):
                    nc.gpsimd.tensor_mul(
                        prod_g[:, k * NT:(k + 1) * NT],
                        xt_sb[:, k * NT:(k + 1) * NT],
                        e_sb[:],
                    )
                for k in range(GP_CHUNKS, KCH):
                    nc.vector.scalar_tensor_tensor(
                        scr[:],
                        xt_sb[:, k * NT:(k + 1) * NT],
                        1.0,
                        e_sb[:],
                        op0=OP.mult,
                        op1=OP.mult,
                        accum_out=racc[:, k * TILES + t: k * TILES + t + 1],
                    )
                return (s, t, prod_g)

            def emit_gp_accum(s, t, prod_g):
                racc = state[s]
                scr_a = scrap.tile([128, NT], bf)
                for k in range(GP_CHUNKS):
                    nc.scalar.activation(
                        scr_a[:],
                        prod_g[:, k * NT:(k + 1) * NT],
                        AF.Copy,
                        accum_out=racc[:, k * TILES + t: k * TILES + t + 1],
                    )
                if t == TILES - 1:
                    out_sb = outsbp.tile([128, KCH], f32)
                    nc.vector.reduce_sum(
                        out_sb[:],
                        racc[:].rearrange("p (k t) -> p k t", k=KCH),
                        axis=mybir.AxisListType.X,
                    )
                    nc.gpsimd.dma_start(outp[s], out_sb[:])

            for g in range(SPC * TILES):
                s, t = divmod(g, TILES)
                if t == 0:
                    state[s] = raccp.tile(
                        [128, KCH * TILES], f32, tag=f"racc{s}", name=f"racc{s}"
                    )
                xt_sb = xtp.tile([128, KCH * NT], bf)
                nc.sync.dma_start(xt_sb[:], xt[s, t])
                h_ps = hpsp.tile([128, HH * NT], f32, name="h_ps")
                for half in range(HH):
                    for k in range(KCH):
                        nc.tensor.matmul(
                            h_ps[:, half * NT:(half + 1) * NT],
                            w1t_sb[:, k * H + half * 128: k * H + half * 128 + 128],
                            xt_sb[:, k * NT:(k + 1) * NT],
                            start=(k == 0), stop=(k == KCH - 1),
                        )
                tanh_sb = tp.tile([128, HH * NT], bf)
                for half in range(HH):
                    nc.scalar.activation(
                        tanh_sb[:, half * NT:(half + 1) * NT],
                        h_ps[:, half * NT:(half + 1) * NT], AF.Tanh,
                    )
                if prev_gp is not None:
                    emit_gp_accum(*prev_gp)
                    prev_gp = None
                if prev is not None:
                    prev_gp = emit_scores_and_wsum(*prev)
                prev = (s, t, xt_sb, tanh_sb)
            emit_gp_accum(*prev_gp)
            prev_gp = emit_scores_and_wsum(*prev)
            emit_gp_accum(*prev_gp)

    nc.compile()
    return nc


def _get_nc():
    if "nc" not in _NC_CACHE:
        _NC_CACHE["nc"] = _build_nc()
    return _NC_CACHE["nc"]


def _prep_inputs(tiles_embeddings, W1, W2):
    X_bf = tiles_embeddings.astype(BF16)
    # xt[b, t, p, k, j] = X[b, t*NT + j, k*128 + p]
    xt_sw = np.ascontiguousarray(
        X_bf.reshape(B, TILES, NT, KCH, 128).transpose(0, 1, 4, 3, 2)
    ).reshape(B, TILES, 128, KCH * NT)
    # w1t[p, k, h] = W1[h, k*128 + p]
    w1t = np.ascontiguousarray(
        W1.astype(BF16).reshape(H, KCH, 128).transpose(2, 1, 0)
    ).reshape(128, KCH * H)
    # w2rep[p, half*128 + c] = W2[0, half*128 + p]
    w2rep = np.ascontiguousarray(
        np.broadcast_to(
            W2.astype(BF16).reshape(HH, 128).transpose(1, 0)[:, :, None],
            (128, HH, 128),
        )
    ).reshape(128, H)
    return [
        {
            "xt": xt_sw[c * SPC:(c + 1) * SPC],
            "w1t": w1t,
            "w2rep": w2rep,
        }
        for c in range(NCORES)
    ]


def _run(tiles_embeddings, W1, W2, **spmd_kwargs):
    nc = _get_nc()
    in_maps = _prep_inputs(tiles_embeddings, W1, W2)
    res = run_bass_kernel_spmd(nc, in_maps, core_ids=list(range(NCORES)), **spmd_kwargs)
    acc = np.concatenate([r["out"] for r in res.results], axis=0)       # [B, 128, KCH]
    e = np.concatenate([r["oute"] for r in res.results], axis=0)        # [B, TILES, NT]
    l = e.astype(np.float64).sum(axis=(1, 2))                           # [B]
    # out[b, k*128 + p] = acc[b, p, k]
    out = acc.transpose(0, 2, 1).reshape(B, D) / l[:, None]
    return out.astype(np.float32, copy=False), res


def kernel(tiles_embeddings, W1, W2):
    out, _ = _run(
        np.asarray(tiles_embeddings), np.asarray(W1), np.asarray(W2)
    )
    return out


# revision 21
# speedup vs baseline: 1.2803x; 1.2803x over previous
"""AttentionMIL pooling kernel for 8 Trainium2 NeuronCores.

Math (per slide b): h = tanh(X @ W1^T); s = h @ w2; a = softmax(s);
out = a^T @ X, with X [N=8192, D=1024], W1 [H=256, D], w2 [H].

Strategy (v2 — single X copy, wsum on the vector engine):
  - Data-parallel over the slide dim: 16 slides / 8 cores = 2 per core.
  - ONE host-swizzled transposed copy of X per core (bf16, [128(d-chunk),
    n-free] tiles) — 32 MiB of HBM traffic per core instead of the 64 MiB
    the two-layout variant needed.
  - Scores in h^T orientation: for each 512-row n-tile, PE computes
    hT[half] [128, 512] = w1t_chunk^T @ xt_chunk accumulated over the 8
    d-chunks (N=512 moving operand — best PE efficiency), ACT applies
    tanh -> bf16, then two more PE matmuls with a REPLICATED w2 stationary
    ([128, 128] with every column equal to w2-half) produce the scores
    already broadcast across all 128 partitions; ACT exp -> e128 bf16.
  - Softmax without a max pass: s = w2 . tanh(.) is bounded by ||w2||_1
    (~13 for this data), so exp(s) cannot overflow fp32.
  - Weighted sum OFF the tensor engine: DVE tensor_tensor_reduce computes
    acc[d-chunk] += sum_n xt[d, n] * e128[d, n] per tile (fused multiply+
    reduce over the free dim, one bf16 2x-mode pass), accumulating into
    per-tile columns of an SBUF tile; one final reduce folds the 16 tile
    columns. l = sum(e) via a 1-lane reduce of e128 row 0 per tile.
  - out = acc / l on host (tiny).
"""

import sys

sys.path.insert(0, "/opt/trn_rl_repo")

import numpy as np
import ml_dtypes

import concourse.bacc as bacc
import concourse.tile as tile
from concourse import mybir
from concourse.bass_utils import run_bass_kernel_spmd

BF16 = ml_dtypes.bfloat16
B, N, D, H = 16, 8192, 1024, 256
NCORES = 8
SPC = B // NCORES          # slides per core
NT = 512                   # rows of N per tile
TILES = N // NT
KCH = D // 128             # d-chunks (contraction chunks of 128)
HH = H // 128              # h halves
GP_CHUNKS = 1              # d-chunks routed GpSimd(mul) + Scalar(accum)

_NC_CACHE = {}


def _build_nc():
    bf = mybir.dt.bfloat16
    f32 = mybir.dt.float32
    AF = mybir.ActivationFunctionType
    OP = mybir.AluOpType

    nc = bacc.Bacc("TRN2", num_devices=NCORES)
    # Host-swizzled transposed layout: each per-tile DMA reads one fully
    # contiguous 1 MiB region into a [128, free] SBUF tile.
    #   xt[s, t, p, k*NT + j] = X[s, t*NT + j, k*128 + p]
    xt = nc.declare_dram_parameter("xt", [SPC, TILES, 128, KCH * NT], bf, isOutput=False)
    # w1t[p, k*H + h] = W1[h, k*128 + p]
    w1t = nc.declare_dram_parameter("w1t", [128, KCH * H], bf, isOutput=False)
    # w2rep[p, half*128 + c] = W2[0, half*128 + p]  (replicated along c)
    w2rep = nc.declare_dram_parameter("w2rep", [128, H], bf, isOutput=False)
    outp = nc.declare_dram_parameter("out", [SPC, 128, KCH], f32, isOutput=True)
    # e row per tile, summed on host for the softmax denominator
    oute = nc.declare_dram_parameter("oute", [SPC, TILES, NT], bf, isOutput=True)

    with tile.TileContext(nc) as tc:
        with tc.tile_pool(name="const", bufs=1) as constp, \
             tc.tile_pool(name="xt", bufs=6) as xtp, \
             tc.tile_pool(name="tanh", bufs=3) as tp, \
             tc.tile_pool(name="e128", bufs=3) as ep, \
             tc.tile_pool(name="scr", bufs=2) as scrp, \
             tc.tile_pool(name="scra", bufs=2) as scrap, \
             tc.tile_pool(name="gprod", bufs=3) as gprodp, \
             tc.tile_pool(name="racc", bufs=2) as raccp, \
             tc.tile_pool(name="outsb", bufs=2) as outsbp, \
             tc.tile_pool(name="hps", bufs=2, space="PSUM") as hpsp, \
             tc.tile_pool(name="sps", bufs=2, space="PSUM") as spsp, \
             tc.tile_pool(name="warm", bufs=1, space="PSUM") as warmp:

            w1t_sb = constp.tile([128, KCH * H], bf)
            nc.gpsimd.dma_start(w1t_sb[:], w1t[:, :])
            w2r_sb = constp.tile([128, H], bf)
            nc.gpsimd.dma_start(w2r_sb[:], w2rep[:, :])

            warm_sb = constp.tile([128, 256], bf)
            nc.gpsimd.memset(warm_sb[:], 0.0)
            warm_ps = warmp.tile([128, 256], f32)
            for _ in range(28):
                nc.tensor.matmul(
                    warm_ps[:, 0:H], warm_sb[:, 0:128], warm_sb[:, 0:H],
                    start=True, stop=True, skip_group_check=True,
                )

            state = {}          # per-slide persistent tiles
            prev = None         # (s, t, xt_sb, tanh_sb)
            prev_gp = None      # (s, t, prod_g)

            def emit_scores_and_wsum(s, t, xt_sb, tanh_sb):
                # scores: two matmuls with replicated-w2 stationary ->
                # s_ps [128, 512] (every partition = the score row)
                s_ps = spsp.tile([128, NT], f32)
                for half in range(HH):
                    nc.tensor.matmul(
                        s_ps[:],
                        w2r_sb[:, half * 128:(half + 1) * 128],
                        tanh_sb[:, half * NT:(half + 1) * NT],
                        start=(half == 0), stop=(half == HH - 1),
                    )
                e_sb = ep.tile([128, NT], bf)
                nc.scalar.activation(e_sb[:], s_ps[:], AF.Exp)
                nc.sync.dma_start(oute[s, t:t + 1, :], e_sb[0:1, :])

                racc = state[s]
                scr = scrp.tile([128, NT], bf)
                # Fused STT runs at 1x on DVE (bf16 2x is stock-op only), so
                # DVE alone (8 x 613ns) can't keep up with PE (3.84us/tile).
                # Ship GP_CHUNKS d-chunks to GpSimd (multiply) + Scalar
                # (Copy-with-accum reduce); DVE keeps the rest fused.
                prod_g = gprodp.tile([128, GP_CHUNKS * NT], bf)
                for k in range(GP_CHUNKS):
                    nc.gpsimd.tensor_mul(
                        prod_g[:, k * NT:(k + 1) * NT],
                        xt_sb[:, k * NT:(k + 1) * NT],
                        e_sb[:],
                    )
                for k in range(GP_CHUNKS, KCH):
                    nc.vector.scalar_tensor_tensor(
                        scr[:],
                        xt_sb[:, k * NT:(k + 1) * NT],
                        1.0,
                        e_sb[:],
                        op0=OP.mult,
                        op1=OP.mult,
                        accum_out=racc[:, k * TILES + t: k * TILES + t + 1],
                    )
                return (s, t, prod_g)

            def emit_gp_accum(s, t, prod_g):
                racc = state[s]
                scr_a = scrap.tile([128, NT], bf)
                for k in range(GP_CHUNKS):
                    nc.scalar.activation(
                        scr_a[:],
                        prod_g[:, k * NT:(k + 1) * NT],
                        AF.Copy,
                        accum_out=racc[:, k * TILES + t: k * TILES + t + 1],
                    )
                if t == TILES - 1:
                    out_sb = outsbp.tile([128, KCH], f32)
                    nc.vector.reduce_sum(
                        out_sb[:],
                        racc[:].rearrange("p (k t) -> p k t", k=KCH),
                        axis=mybir.AxisListType.X,
                    )
                    nc.gpsimd.dma_start(outp[s], out_sb[:])

            for g in range(SPC * TILES):
                s, t = divmod(g, TILES)
                if t == 0:
                    state[s] = raccp.tile(
                        [128, KCH * TILES], f32, tag=f"racc{s}", name=f"racc{s}"
                    )
                xt_sb = xtp.tile([128, KCH * NT], bf)
                nc.sync.dma_start(xt_sb[:], xt[s, t])
                h_ps = hpsp.tile([128, HH * NT], f32, name="h_ps")
                for half in range(HH):
                    for k in range(KCH):
                        nc.tensor.matmul(
                            h_ps[:, half * NT:(half + 1) * NT],
                            w1t_sb[:, k * H + half * 128: k * H + half * 128 + 128],
                            xt_sb[:, k * NT:(k + 1) * NT],
                            start=(k == 0), stop=(k == KCH - 1),
                        )
                tanh_sb = tp.tile([128, HH * NT], bf)
                for half in range(HH):
                    nc.scalar.activation(
                        tanh_sb[:, half * NT:(half + 1) * NT],
                        h_ps[:, half * NT:(half + 1) * NT], AF.Tanh,
                    )
                if prev_gp is not None:
                    emit_gp_accum(*prev_gp)
                    prev_gp = None
                if prev is not None:
                    prev_gp = emit_scores_and_wsum(*prev)
                prev = (s, t, xt_sb, tanh_sb)
            emit_gp_accum(*prev_gp)
            prev_gp = emit_scores_and_wsum(*prev)
            emit_gp_accum(*prev_gp)

    nc.compile()
    return nc


def _get_nc():
    if "nc" not in _NC_CACHE:
        _NC_CACHE["nc"] = _build_nc()
    return _NC_CACHE["nc"]


def _prep_inputs(tiles_embeddings, W1, W2):
    X_bf = tiles_embeddings.astype(BF16)
    # xt[b, t, p, k, j] = X[b, t*NT + j, k*128 + p]
    xt_sw = np.ascontiguousarray(
        X_bf.reshape(B, TILES, NT, KCH, 128).transpose(0, 1, 4, 3, 2)
    ).reshape(B, TILES, 128, KCH * NT)
    # w1t[p, k, h] = W1[h, k*128 + p]
    w1t = np.ascontiguousarray(
        W1.astype(BF16).reshape(H, KCH, 128).transpose(2, 1, 0)
    ).reshape(128, KCH * H)
    # w2rep[p, half*128 + c] = W2[0, half*128 + p]
    w2rep = np.ascontiguousarray(
        np.broadcast_to(
            W2.astype(BF16).reshape(HH, 128).transpose(1, 0)[:, :, None],
            (128, HH, 128),
        )
    ).reshape(128, H)
    return [
        {
            "xt": xt_sw[c * SPC:(c + 1) * SPC],
            "w1t": w1t,
            "w2rep": w2rep,
        }
        for c in range(NCORES)
    ]


def _run(tiles_embeddings, W1, W2, **spmd_kwargs):
    nc = _get_nc()
    in_maps = _prep_inputs(tiles_embeddings, W1, W2)
    res = run_bass_kernel_spmd(nc, in_maps, core_ids=list(range(NCORES)), **spmd_kwargs)
    acc = np.concatenate([r["out"] for r in res.results], axis=0)       # [B, 128, KCH]
    e = np.concatenate([r["oute"] for r in res.results], axis=0)        # [B, TILES, NT]
    l = e.astype(np.float64).sum(axis=(1, 2))                           # [B]
    # out[b, k*128 + p] = acc[b, p, k]
    out = acc.transpose(0, 2, 1).reshape(B, D) / l[:, None]
    return out.astype(np.float32, copy=False), res


def kernel(tiles_embeddings, W1, W2):
    out, _ = _run(
        np.asarray(tiles_embeddings), np.asarray(W1), np.asarray(W2)
    )
    return out
